# revision 1
# baseline (speedup 1.0000x reference)
"""Trainium2 Bass kernel for nn_DCAM (dense transformer attention module).

Reference computation (per batch b):
  qp/kp/vp = avg_pool2d(feature_{q,k,v}, 2)            # (C=256, 64, 64)
  q = Wq @ qp, k = Wk @ kp  (M=32 channels)            # (32, N=4096)
  v = Wv @ vp                                          # (256, N)
  attn = softmax(q^T k, axis=-1)                       # (N, N)
  out[c, m] = sum_n v[c, n] attn[m, n]                 # (256, N)
  result = upsample_nearest(out, 2) + feature_v        # (256, 128, 128)

Sharding: data-parallel over batch B=8 across 8 NeuronCores (1 batch/core).

Per-core design notes:
  - S^T computed directly (lhsT = k j-block, rhs = q i-chunk) so the
    softmax denominator and the output matmul need no transposes.
  - All hot matmuls in bf16 (fp32r runs at ~2 cyc/row and keeps the PE
    HAM clock-gate cold). The precision-critical S path uses a hi/lo
    bf16 split: s = qh*kh + qh*kl + ql*kh (error ~2^-17).
  - S matmuls are K=32, so 4 j-blocks run concurrently in the PE array
    via tile_position row tiling (k at partition groups 0/32/64/96,
    q replicated into all four groups).
  - S psum/P tiles are paired (128, 1024) - two j-blocks side by side -
    halving ACT/DVE instruction counts. The denominator accumulators are
    also (128, 1024) (independent halves merged at i-chunk end).
  - vertical 2x2-pooling pairs are summed by the DMA itself
    (SWDGE accum_op=add); only the horizontal add runs on DVE.
  - softmax without max-subtraction (|s| <= ~15 fits fp32 easily).
  - denominator -> 1/l via DVE reciprocal on a (128, 4) view
    (DRAM-bounce transpose); broadcast back via DMA. No Ln, so a single
    ACT table set (Copy+Exp) is loaded exactly once.
  - feature_v kept resident in SBUF as bf16 for the final residual add.
  - pooling is a 2x2 *sum*; scales fold into the exp scale (1/16) and
    into WvT (x0.25) on the host.
"""
import numpy as np
import ml_dtypes

import concourse.bass as bass
import concourse.mybir as mybir
import concourse.tile as tile
from concourse import bacc
from concourse.bass_utils import run_bass_kernel_spmd

F32 = mybir.dt.float32
F32R = mybir.dt.float32r
BF16 = mybir.dt.bfloat16
AF = mybir.ActivationFunctionType
ADD = mybir.AluOpType.add

B = 8
C = 256
M = 32
H = W = 128
HP = WP = 64
N = HP * WP          # 4096
CB = C // 128        # 2 channel blocks
JB = N // 128        # 32 key blocks
JG = JB // 4         # 8 groups of 4 packed j-blocks
IC = N // 512        # 8 query chunks


def build_module():
    nc = bacc.Bacc("TRN2", target_bir_lowering=False, debug=False)

    fq_d = nc.dram_tensor("feature_q", [C, H, W], F32, kind="ExternalInput").ap()
    fk_d = nc.dram_tensor("feature_k", [C, H, W], F32, kind="ExternalInput").ap()
    fv_d = nc.dram_tensor("feature_v", [C, H, W], F32, kind="ExternalInput").ap()
    wqh_d = nc.dram_tensor("WqTh", [C, M], BF16, kind="ExternalInput").ap()
    wql_d = nc.dram_tensor("WqTl", [C, M], BF16, kind="ExternalInput").ap()
    wkh_d = nc.dram_tensor("WkTh", [C, M], BF16, kind="ExternalInput").ap()
    wkl_d = nc.dram_tensor("WkTl", [C, M], BF16, kind="ExternalInput").ap()
    wvt_d = nc.dram_tensor("WvT", [C, C], BF16, kind="ExternalInput").ap()
    out_d = nc.dram_tensor("out", [C, H, W], F32, kind="ExternalOutput").ap()

    with tile.TileContext(nc) as tc:
        with tc.tile_pool(name="const", bufs=1) as cpool, \
             tc.tile_pool(name="persist", bufs=1) as pp, \
             tc.tile_pool(name="ps", bufs=1, space="PSUM") as ps, \
             tc.tile_pool(name="dramb", bufs=2, space="DRAM") as dpool:
            # ---- constants ----
            w_sb = {}
            for nm, dram in (("qh", wqh_d), ("ql", wql_d),
                             ("kh", wkh_d), ("kl", wkl_d)):
                t = cpool.tile([128, CB, M], BF16, name=f"w_{nm}")
                nc.sync.dma_start(t[:], dram.rearrange("(b p) m -> p b m", p=128))
                w_sb[nm] = t
            wv_sb = cpool.tile([128, CB, C], BF16)
            nc.sync.dma_start(wv_sb[:], wvt_d.rearrange("(b p) c -> p b c", p=128))
            ones_col = cpool.tile([128, 1], F32R)
            nc.vector.memset(ones_col.bitcast(F32), 1.0)

            # ---- persistent tensors ----
            q4h = pp.tile([128, N], BF16)             # q hi replicated x4
            q4l = pp.tile([128, N], BF16)             # q lo replicated x4
            kh_all = pp.tile([128, JG, 128], BF16)    # [32*(jb%4)+m, jb//4, jf]
            kl_all = pp.tile([128, JG, 128], BF16)
            vt_all = pp.tile([128, JB, C], BF16)      # vT[j, c] per j-block
            fv_sb = pp.tile([128, CB, H, W], BF16)    # resident residual copy

            # fv load early: stream alongside phase A1 (casts f32->bf16)
            for cb in range(CB):
                for hh in range(2):
                    nc.gpsimd.dma_start(
                        fv_sb[:, cb, hh * 64:(hh + 1) * 64, :],
                        fv_d[cb * 128:(cb + 1) * 128,
                             hh * 64:(hh + 1) * 64, :])

            # =========== Phase A2: pool fv, project vT ===========
            with tc.tile_pool(name="poolV", bufs=1) as pv:
                for half in range(2):  # 32 pooled rows each
                    vph = pv.tile([128, CB, 32, WP], BF16, tag="vph", bufs=2,
                                  name="vph")
                    for cb in range(CB):
                        for sub in range(2):  # 16 pooled rows
                            raw0 = half * 64 + sub * 32
                            src = fv_sb[:, cb, raw0:raw0 + 32, :].rearrange(
                                "c (h dy) (w dx) -> c h dy w dx", dy=2, dx=2)
                            rfv = pv.tile([128, 16, WP, 2], BF16, tag="rfv",
                                          bufs=3, name="rfv")
                            nc.gpsimd.tensor_add(rfv[:], src[:, :, 0],
                                                 src[:, :, 1])
                            nc.gpsimd.tensor_add(
                                vph[:, cb, sub * 16:(sub + 1) * 16, :],
                                rfv[:, :, :, 0], rfv[:, :, :, 1])
                    for r2 in range(16):  # j-blocks in this half
                        jb = half * 16 + r2
                        vt_ps = ps.tile([128, 512], F32, tag="o1",
                                        bufs=2, name="vt_ps")[:, :C]
                        nc.tensor.matmul(vt_ps[:],
                                         vph[:, 0, r2 * 2:r2 * 2 + 2, :],
                                         wv_sb[:, 0], start=True, stop=False)
                        nc.tensor.matmul(vt_ps[:],
                                         vph[:, 1, r2 * 2:r2 * 2 + 2, :],
                                         wv_sb[:, 1], start=False, stop=True)
                        nc.scalar.copy(vt_all[:, jb, :], vt_ps[:])

            # =========== Phase A1: pool + project q and k ===========
            with tc.tile_pool(name="poolA", bufs=1) as pa:
                for ti, feat in ((0, fq_d), (1, fk_d)):
                    # per-tensor tags -> the fq and fk pipelines overlap;
                    # qp is a rolling per-chunk buffer, projected immediately.
                    nm = "q" if ti == 0 else "k"
                    wh = w_sb["qh" if ti == 0 else "kh"]
                    wl = w_sb["ql" if ti == 0 else "kl"]
                    for icn in range(IC):  # 8 pooled rows = one i-chunk
                        qp_h = pa.tile([128, CB, 8, WP], BF16,
                                       tag=f"qp_h{nm}", bufs=3, name="qp_h")
                        qp_l = pa.tile([128, CB, 8, WP], BF16,
                                       tag=f"qp_l{nm}", bufs=3, name="qp_l")
                        for cb in range(CB):
                            x5 = pa.tile([128, 8, 2, WP, 2], F32,
                                         tag=f"x{nm}", bufs=2, name="x5")
                            src = feat[cb * 128:(cb + 1) * 128,
                                       icn * 16:(icn + 1) * 16, :]
                            nc.sync.dma_start(
                                x5[:],
                                src.rearrange("c (h dy) (w dx) -> c h dy w dx",
                                              dy=2, dx=2))
                            r = pa.tile([128, 8, WP, 2], F32, tag=f"r{nm}",
                                        bufs=2, name="r")
                            nc.vector.tensor_add(r[:], x5[:, :, 0], x5[:, :, 1])
                            qpc = pa.tile([128, 8, WP], F32, tag=f"qpc{nm}",
                                          bufs=2, name="qpc")
                            nc.vector.tensor_add(qpc[:], r[:, :, :, 0],
                                                 r[:, :, :, 1])
                            nc.scalar.copy(qp_h[:, cb], qpc[:])
                            nc.vector.tensor_sub(qp_l[:, cb], qpc[:],
                                                 qp_h[:, cb])
                        pr_ps = ps.tile([128, 512], F32,
                                        tag="o0" if ti == 0 else "o1",
                                        bufs=2, name="pr_ps")[:M, :]
                        mms = [(wt, qt, cb)
                               for cb in range(CB)
                               for (wt, qt) in ((wh, qp_h), (wh, qp_l),
                                                (wl, qp_h))]
                        for mi, (wt, qt, cb) in enumerate(mms):
                            nc.tensor.matmul(pr_ps[:], wt[:, cb],
                                             qt[:, cb],
                                             start=(mi == 0),
                                             stop=(mi == len(mms) - 1),
                                             skip_group_check=True)
                        # evict hi/lo; k goes into the packed layout
                        if ti == 0:
                            cs = slice(icn * 512, (icn + 1) * 512)
                            nc.scalar.copy(q4h[0:32, cs], pr_ps[:])
                            nc.vector.tensor_sub(q4l[0:32, cs], pr_ps[:],
                                                 q4h[0:32, cs])
                            # replicate this chunk into partition groups
                            # 1..3 immediately, so attention for this
                            # i-chunk can start without waiting for all
                            # of feature_q.
                            for g in range(1, 4):
                                gs = slice(g * 32, (g + 1) * 32)
                                nc.sync.dma_start(q4h[gs, cs],
                                                 q4h[0:32, cs])
                                nc.sync.dma_start(q4l[gs, cs],
                                                 q4l[0:32, cs])
                        else:
                            for t in range(4):
                                pslc = pr_ps[:, t * 128:(t + 1) * 128]
                                nc.scalar.copy(kh_all[t * 32:(t + 1) * 32,
                                                      icn, :], pslc)
                                nc.vector.tensor_sub(
                                    kl_all[t * 32:(t + 1) * 32, icn, :],
                                    pslc, kh_all[t * 32:(t + 1) * 32, icn, :])

            # =========== Phase B: attention + fused epilogue ===========
            TERMS = ((0, 0), (0, 1), (1, 0))  # (k hi/lo, q hi/lo)
            with tc.tile_pool(name="poolB", bufs=1) as pb:
                for ic in range(IC):
                    i0 = ic * 512
                    lacc_d = pb.tile([128, 1024], F32R, tag="lacc_d", bufs=2,
                                     name="lacc_d")
                    lacc_g = pb.tile([128, 1024], F32R, tag="lacc_g", bufs=2,
                                     name="lacc_g")
                    nc.vector.memset(lacc_d.bitcast(F32), 0.0)
                    nc.gpsimd.memset(lacc_g.bitcast(F32), 0.0)
                    o_ps = [ps.tile([128, 512], F32, tag=f"o{cb}", bufs=2,
                                     name=f"o{cb}_ps")
                            for cb in range(CB)]
                    for jg in range(JG):
                        s_ps = [ps.tile([128, 1024], F32, tag=f"s{u}",
                                         bufs=1, name=f"s{u}_ps")
                                for u in range(2)]
                        for t in range(4):
                            gs = slice(t * 32, (t + 1) * 32)
                            dst = s_ps[t // 2][:, (t % 2) * 512:
                                               (t % 2) * 512 + 512]
                            for term, (kk, qq) in enumerate(TERMS):
                                ka = kh_all if kk == 0 else kl_all
                                qa = q4h if qq == 0 else q4l
                                nc.tensor.matmul(
                                    dst, ka[gs, jg, :], qa[gs, i0:i0 + 512],
                                    start=(term == 0), stop=(term == 2),
                                    tile_position=(t * 32, 0),
                                    skip_group_check=True)
                        p_t = []
                        for u in range(2):
                            p = pb.tile([128, 1024], BF16, tag="p", bufs=8,
                                        name="p")
                            nc.scalar.activation(p[:], s_ps[u][:], AF.Exp,
                                                 scale=0.0625)
                            p_t.append(p)
                        nc.vector.tensor_add(lacc_d[:], lacc_d[:], p_t[0][:])
                        nc.gpsimd.tensor_add(lacc_g[:], lacc_g[:], p_t[1][:])
                        for u in range(2):
                            for tt in range(2):
                                j = jg * 4 + u * 2 + tt
                                pr = p_t[u][:, tt * 512:tt * 512 + 512]
                                for cb in range(CB):
                                    nc.tensor.matmul(
                                        o_ps[cb][:],
                                        vt_all[:, j, cb * 128:(cb + 1) * 128],
                                        pr,
                                        start=(j == 0), stop=(j == JB - 1),
                                        skip_group_check=True)
                    # ---- fused epilogue for this i-chunk ----
                    # l = column sums of all four accumulator halves, merged
                    # for free by PSUM accumulation across four ones-matmuls.
                    l_ps = ps.tile([128, 1024], F32, tag="s0", bufs=1,
                                    name="l_ps")
                    halves = [lacc_d[:, :512], lacc_d[:, 512:],
                              lacc_g[:, :512], lacc_g[:, 512:]]
                    for hi_, hv in enumerate(halves):
                        nc.tensor.matmul(l_ps[:1, :512], ones_col[:], hv,
                                         start=(hi_ == 0),
                                         stop=(hi_ == len(halves) - 1),
                                         skip_group_check=True)
                    l_sb = pb.tile([1, 512], F32, tag="l_sb", bufs=2,
                                   name="l_sb")
                    nc.scalar.copy(l_sb[:], l_ps[:1, :512])
                    # transpose to (128, 4) via DRAM bounce, reciprocal, back
                    l_dr = dpool.tile([512], F32, tag="l_dr", bufs=2,
                                      name="l_dr")
                    nc.sync.dma_start(l_dr[:], l_sb[:])
                    lT = pb.tile([128, 4], F32, tag="lT", bufs=2, name="lT")
                    nc.sync.dma_start(lT[:], l_dr.rearrange("(p b) -> p b",
                                                            b=4))
                    rT = pb.tile([128, 4], F32, tag="rT", bufs=2, name="rT")
                    nc.vector.reciprocal(rT[:], lT[:])
                    r_dr = dpool.tile([512], F32, tag="r_dr", bufs=2,
                                      name="r_dr")
                    nc.sync.dma_start(r_dr.rearrange("(p b) -> p b", b=4),
                                      rT[:])
                    rb_sb = pb.tile([128, 512], F32, tag="rb_sb", bufs=2,
                                    name="rb_sb")
                    nc.sync.dma_start(
                        rb_sb[:],
                        r_dr.rearrange("(o x) -> o x", o=1).to_broadcast(
                            (128, 512)))
                    for cb in range(CB):
                        oc = pb.tile([128, 512], F32, tag="oc", bufs=4,
                                     name="oc")
                        nc.vector.tensor_mul(oc[:], o_ps[cb][:], rb_sb[:])
                        final = pb.tile([128, 8, 2, WP, 2], F32, tag="final",
                                        bufs=3, name="final")
                        up = oc.rearrange("c (h w) -> c h w", w=WP)[
                            :, :, :, None].to_broadcast((128, 8, WP, 2))
                        fvv = fv_sb[:, cb, ic * 16:(ic + 1) * 16, :].rearrange(
                            "c (h dy) (w dx) -> c h dy w dx", dy=2, dx=2)
                        nc.vector.tensor_add(final[:, :, 0], up, fvv[:, :, 0])
                        nc.vector.tensor_add(final[:, :, 1], up, fvv[:, :, 1])
                        nc.sync.dma_start(
                            out_d[cb * 128:(cb + 1) * 128,
                                  ic * 16:(ic + 1) * 16, :],
                            final.rearrange("c h dy w dx -> c (h dy) (w dx)"))

    nc.compile()
    return nc


_NC_CACHE = []
LAST_RESULT = []  # last BassKernelResults, for perf inspection by test.py


def _bf16_split(x):
    hi = x.astype(ml_dtypes.bfloat16)
    lo = (x - hi.astype(np.float32)).astype(ml_dtypes.bfloat16)
    return np.ascontiguousarray(hi), np.ascontiguousarray(lo)


def kernel(**inputs) -> np.ndarray:
    fq = np.ascontiguousarray(np.asarray(inputs["feature_q"], dtype=np.float32))
    fk = np.ascontiguousarray(np.asarray(inputs["feature_k"], dtype=np.float32))
    fv = np.ascontiguousarray(np.asarray(inputs["feature_v"], dtype=np.float32))
    wq = np.asarray(inputs["Wq"], dtype=np.float32)
    wk = np.asarray(inputs["Wk"], dtype=np.float32)
    wv = np.asarray(inputs["Wv"], dtype=np.float32)

    # weight layout prep (pure layout/scale folding, no heavy compute):
    # on-device pooling is a 2x2 *sum*; q,k each pick up 4x -> s is 16x,
    # folded into the on-device exp scale; v's 4x is folded into WvT here.
    wqh, wql = _bf16_split(wq.T)                      # (C, M) hi/lo
    wkh, wkl = _bf16_split(wk.T)
    wvt = np.ascontiguousarray(
        (wv.T * 0.25).astype(ml_dtypes.bfloat16))     # (C, C) [c_in, c_out]

    if not _NC_CACHE:
        _NC_CACHE.append(build_module())
    nc = _NC_CACHE[0]

    in_maps = [
        {
            "feature_q": fq[b],
            "feature_k": fk[b],
            "feature_v": fv[b],
            "WqTh": wqh,
            "WqTl": wql,
            "WkTh": wkh,
            "WkTl": wkl,
            "WvT": wvt,
        }
        for b in range(B)
    ]
    res = run_bass_kernel_spmd(nc, in_maps, core_ids=list(range(B)))
    LAST_RESULT.clear()
    LAST_RESULT.append(res)
    out = np.stack([res.results[b]["out"] for b in range(B)], axis=0)
    return out.astype(np.float32)


if __name__ == "__main__":
    nc = build_module()
    print("module built + compiled OK")



# revision 3
# speedup vs baseline: 1.3249x; 1.3249x over previous
"""Trainium2 Bass kernel for nn_DCAM (dense transformer attention module).

Reference computation (per batch b):
  qp/kp/vp = avg_pool2d(feature_{q,k,v}, 2)            # (C=256, 64, 64)
  q = Wq @ qp, k = Wk @ kp  (M=32 channels)            # (32, N=4096)
  v = Wv @ vp                                          # (256, N)
  attn = softmax(q^T k, axis=-1)                       # (N, N)
  out[c, m] = sum_n v[c, n] attn[m, n]                 # (256, N)
  result = upsample_nearest(out, 2) + feature_v        # (256, 128, 128)

Sharding: data-parallel over batch B=8 across 8 NeuronCores (1 batch/core).

Per-core design (v2 - single-precision bf16, pooling folded into PE):
  - Inputs stream HBM->SBUF as f32->bf16 cast DMAs (SWDGE). 2x2 sum-pooling
    is never computed by DVE/GPSIMD: for q/k it's folded into the projection
    matmuls as 4 PSUM-accumulated terms with stride-2 access patterns; for v,
    vertical pairs are pre-summed by one DVE pass and the horizontal fold
    happens in the vT projection (2 strided lhsT terms per cb).
  - Single bf16 term for S and all projections (measured end-to-end rel err
    ~3e-3 vs the 2e-2 gate; hi/lo splits buy nothing).
  - S^T computed directly (lhsT = k j-block, rhs = q i-chunk); K=32 so 4
    j-blocks run concurrently via tile_position row tiling.
  - softmax denominator: bf16 pairwise tree over the 16 P tiles per i-chunk
    on DVE (2 adds on GPSIMD), merged by a 2-matmul ones reduction into PSUM.
  - exp on ACT from PSUM at FD=1024; P written bf16 straight to SBUF.
  - Load order: fq chunk0, then fk/fv chunk pairs with i-chunk 0's attention
    interleaved so compute chases the DMA stream; fq 1..7 trickle during the
    remaining i-chunks.
  - pooling is a 2x2 *sum*; q,k scales fold into the exp scale (1/16), v's
    into WvT (x0.25) on the host.
"""
import numpy as np
import ml_dtypes

import concourse.bass as bass
import concourse.mybir as mybir
import concourse.tile as tile
from concourse import bacc
from concourse.bass_utils import run_bass_kernel_spmd

F32 = mybir.dt.float32
BF16 = mybir.dt.bfloat16
AF = mybir.ActivationFunctionType

B = 8
C = 256
M = 32
H = W = 128
HP = WP = 64
N = HP * WP          # 4096
CB = C // 128        # 2 channel blocks
JB = N // 128        # 32 key blocks
JG = JB // 4         # 8 groups of 4 packed j-blocks
IC = N // 512        # 8 query chunks


def build_module():
    nc = bacc.Bacc("TRN2", target_bir_lowering=False, debug=False)

    fq_d = nc.dram_tensor("feature_q", [C, H, W], F32, kind="ExternalInput").ap()
    fk_d = nc.dram_tensor("feature_k", [C, H, W], F32, kind="ExternalInput").ap()
    fv_d = nc.dram_tensor("feature_v", [C, H, W], F32, kind="ExternalInput").ap()
    wqt_d = nc.dram_tensor("WqT", [C, M], BF16, kind="ExternalInput").ap()
    wkt_d = nc.dram_tensor("WkT", [C, M], BF16, kind="ExternalInput").ap()
    wvt_d = nc.dram_tensor("WvT", [C, C], BF16, kind="ExternalInput").ap()
    out_d = nc.dram_tensor("out", [C, H, W], F32, kind="ExternalOutput").ap()

    with tile.TileContext(nc) as tc:
        with tc.tile_pool(name="const", bufs=1) as cpool, \
             tc.tile_pool(name="persist", bufs=1) as pp, \
             tc.tile_pool(name="ps", bufs=1, space="PSUM") as ps, \
             tc.tile_pool(name="work", bufs=1) as pa, \
             tc.tile_pool(name="dramb", bufs=2, space="DRAM") as dpool:
            # ---- constants ----
            wq_sb = cpool.tile([128, CB, M], BF16)
            nc.sync.dma_start(wq_sb[:], wqt_d.rearrange("(b p) m -> p b m", p=128))
            wk_sb = cpool.tile([128, CB, M], BF16)
            nc.sync.dma_start(wk_sb[:], wkt_d.rearrange("(b p) m -> p b m", p=128))
            wv_sb = cpool.tile([128, CB, C], BF16)
            nc.sync.dma_start(wv_sb[:], wvt_d.rearrange("(b p) c -> p b c", p=128))
            ones_b = cpool.tile([128, 1], BF16)
            nc.vector.memset(ones_b[:], 1.0)
            # ACT exp table warm-up (the table load costs ~2.7us; do it now,
            # long before the first real exp).
            dum = cpool.tile([1, 8], F32)
            nc.vector.memset(dum[:], 0.0)
            nc.scalar.activation(dum[:], dum[:], AF.Exp, scale=0.0625)

            # ---- persistent tensors ----
            fv_sb = pp.tile([128, CB, H, W], BF16)    # raw fv (residual + vproj)
            vt_all = pp.tile([128, JB, C], BF16)      # vT[j, c] per j-block
            q4h = pp.tile([128, N], BF16)             # q replicated x4 groups
            kh_all = pp.tile([128, JG, 128], BF16)    # [32*(jb%4)+m, jg, jf]

            # ================= helpers =================
            def q_chunk(icn):
                cq = pa.tile([128, CB, 16, W], BF16, tag="cq", bufs=2,
                             name="cq")
                nc.gpsimd.dma_start(
                    cq[:],
                    fq_d[:, icn * 16:(icn + 1) * 16, :].rearrange(
                        "(b p) h w -> p b h w", p=128))
                pr = ps.tile([128, 512], F32, tag="aux", bufs=2,
                             name="pr_q")[:M, :]
                mm = 0
                for cb in range(CB):
                    cr = cq[:, cb].rearrange("c (h dy) (w dx) -> c h dy w dx",
                                             dy=2, dx=2)
                    for dy in range(2):
                        for dx in range(2):
                            nc.tensor.matmul(pr, wq_sb[:, cb],
                                             cr[:, :, dy, :, dx],
                                             start=(mm == 0), stop=(mm == 7),
                                             skip_group_check=True)
                            mm += 1
                cs = slice(icn * 512, (icn + 1) * 512)
                nc.scalar.copy(q4h[0:32, cs], pr)
                for g in range(1, 4):
                    nc.sync.dma_start(q4h[g * 32:(g + 1) * 32, cs],
                                      q4h[0:32, cs])

            def k_chunk(icn):
                ck = pa.tile([128, CB, 16, W], BF16, tag="ck", bufs=2,
                             name="ck")
                nc.gpsimd.dma_start(
                    ck[:],
                    fk_d[:, icn * 16:(icn + 1) * 16, :].rearrange(
                        "(b p) h w -> p b h w", p=128))
                pr = ps.tile([128, 512], F32, tag="aux", bufs=2,
                             name="pr_k")[:M, :]
                mm = 0
                for cb in range(CB):
                    cr = ck[:, cb].rearrange("c (h dy) (w dx) -> c h dy w dx",
                                             dy=2, dx=2)
                    for dy in range(2):
                        for dx in range(2):
                            nc.tensor.matmul(pr, wk_sb[:, cb],
                                             cr[:, :, dy, :, dx],
                                             start=(mm == 0), stop=(mm == 7),
                                             skip_group_check=True)
                            mm += 1
                for t in range(4):
                    nc.scalar.copy(kh_all[t * 32:(t + 1) * 32, icn, :],
                                   pr[:, t * 128:(t + 1) * 128])

            def v_chunk(icn):
                # load raw chunk into the persistent residual copy
                nc.gpsimd.dma_start(
                    fv_sb[:, :, icn * 16:(icn + 1) * 16, :],
                    fv_d[:, icn * 16:(icn + 1) * 16, :].rearrange(
                        "(b p) h w -> p b h w", p=128))
                # vertical 2x2 pair sums (one DVE pass, bf16 2x mode)
                vv = pa.tile([128, CB, 8, W], BF16, tag="vv", bufs=2,
                             name="vv")
                for cb in range(CB):
                    fvc = fv_sb[:, cb, icn * 16:(icn + 1) * 16, :].rearrange(
                        "c (h dy) w -> c h dy w", dy=2)
                    nc.vector.tensor_add(vv[:, cb], fvc[:, :, 0], fvc[:, :, 1])
                # vT projection per j-block; horizontal fold via 2 strided
                # lhsT terms per cb.
                for r in range(4):
                    jb = icn * 4 + r
                    vt_ps = ps.tile([128, 512], F32, tag="aux", bufs=2,
                                    name="vt_ps")[:, :C]
                    mm = 0
                    for cb in range(CB):
                        vr = vv[:, cb, 2 * r:2 * r + 2, :].rearrange(
                            "c h (w dx) -> c h w dx", dx=2)
                        for dx in range(2):
                            nc.tensor.matmul(vt_ps, vr[:, :, :, dx],
                                             wv_sb[:, cb],
                                             start=(mm == 0), stop=(mm == 3),
                                             skip_group_check=True)
                            mm += 1
                    nc.scalar.copy(vt_all[:, jb, :], vt_ps)

            def b_s_exp(ic, jg, o_ps, t1s):
                i0 = ic * 512
                p_t = []
                for u in range(2):
                    s_u = ps.tile([128, 1024], F32, tag="s", bufs=2,
                                  name="s_u")
                    for h in range(2):
                        t = u * 2 + h
                        gs = slice(t * 32, (t + 1) * 32)
                        nc.tensor.matmul(
                            s_u[:, h * 512:(h + 1) * 512],
                            kh_all[gs, jg, :], q4h[gs, i0:i0 + 512],
                            start=True, stop=True,
                            tile_position=(t * 32, 0),
                            skip_group_check=True)
                    p = pa.tile([128, 1024], BF16, tag="p", bufs=5, name="p")
                    nc.scalar.activation(p[:], s_u[:], AF.Exp, scale=0.0625)
                    p_t.append(p)
                # denominator tree, level 1 (jg 1,3 go to GPSIMD for balance)
                t1 = pa.tile([128, 1024], BF16, tag="t1", bufs=8, name="t1")
                if jg in (1, 3):
                    nc.gpsimd.tensor_add(t1[:], p_t[0][:], p_t[1][:])
                else:
                    nc.vector.tensor_add(t1[:], p_t[0][:], p_t[1][:])
                t1s.append(t1)
                return p_t

            def b_pv(jg, o_ps, p_t):
                for u in range(2):
                    for tt in range(2):
                        j = jg * 4 + u * 2 + tt
                        pr = p_t[u][:, tt * 512:tt * 512 + 512]
                        for cb in range(CB):
                            nc.tensor.matmul(
                                o_ps[cb],
                                vt_all[:, j, cb * 128:(cb + 1) * 128],
                                pr,
                                start=(j == 0), stop=(j == JB - 1),
                                skip_group_check=True)

            def b_tail(ic, o_ps, t1s):
                # finish denominator tree on DVE
                t2s = []
                for i in range(4):
                    t2 = pa.tile([128, 1024], BF16, tag="t2", bufs=4,
                                 name="t2")
                    nc.vector.tensor_add(t2[:], t1s[2 * i][:],
                                         t1s[2 * i + 1][:])
                    t2s.append(t2)
                t3a = pa.tile([128, 1024], BF16, tag="t3", bufs=2, name="t3")
                nc.vector.tensor_add(t3a[:], t2s[0][:], t2s[1][:])
                t3b = pa.tile([128, 1024], BF16, tag="t3", bufs=2, name="t3b")
                nc.vector.tensor_add(t3b[:], t2s[2][:], t2s[3][:])
                tT = pa.tile([128, 1024], BF16, tag="tT", bufs=2, name="tT")
                nc.vector.tensor_add(tT[:], t3a[:], t3b[:])
                # l = column sums of both halves via 2 ones-matmuls
                l_ps = ps.tile([128, 1024], F32, tag="s", bufs=2,
                               name="l_ps")[:1, :512]
                for u in range(2):
                    nc.tensor.matmul(l_ps, ones_b[:],
                                     tT[:, u * 512:(u + 1) * 512],
                                     start=(u == 0), stop=(u == 1),
                                     skip_group_check=True)
                l_sb = pa.tile([1, 512], F32, tag="l_sb", bufs=2, name="l_sb")
                nc.scalar.copy(l_sb[:], l_ps)
                # transpose to (128, 4) via DRAM bounce, reciprocal, back
                l_dr = dpool.tile([512], F32, tag="l_dr", bufs=2, name="l_dr")
                nc.sync.dma_start(l_dr[:], l_sb[:])
                lT = pa.tile([128, 4], F32, tag="lT", bufs=2, name="lT")
                nc.sync.dma_start(lT[:], l_dr.rearrange("(p b) -> p b", b=4))
                rT = pa.tile([128, 4], F32, tag="rT", bufs=2, name="rT")
                nc.vector.reciprocal(rT[:], lT[:])
                r_dr = dpool.tile([512], F32, tag="r_dr", bufs=2, name="r_dr")
                nc.sync.dma_start(r_dr.rearrange("(p b) -> p b", b=4), rT[:])
                rb_sb = pa.tile([128, 512], F32, tag="rb_sb", bufs=2,
                                name="rb_sb")
                nc.sync.dma_start(
                    rb_sb[:],
                    r_dr.rearrange("(o x) -> o x", o=1).to_broadcast(
                        (128, 512)))
                for cb in range(CB):
                    oc = pa.tile([128, 512], F32, tag="oc", bufs=2, name="oc")
                    nc.vector.tensor_mul(oc[:], o_ps[cb][:], rb_sb[:])
                    final = pa.tile([128, 8, 2, WP, 2], F32, tag="final",
                                    bufs=2, name="final")
                    up = oc.rearrange("c (h w) -> c h w", w=WP)[
                        :, :, :, None].to_broadcast((128, 8, WP, 2))
                    fvv = fv_sb[:, cb, ic * 16:(ic + 1) * 16, :].rearrange(
                        "c (h dy) (w dx) -> c h dy w dx", dy=2, dx=2)
                    nc.vector.tensor_add(final[:, :, 0], up, fvv[:, :, 0])
                    nc.gpsimd.tensor_add(final[:, :, 1], up, fvv[:, :, 1])
                    nc.sync.dma_start(
                        out_d[cb * 128:(cb + 1) * 128,
                              ic * 16:(ic + 1) * 16, :],
                        final.rearrange("c h dy w dx -> c (h dy) (w dx)"))

            # ================= schedule =================
            # q chunk 0 first so i-chunk 0 can run during the k/v stream.
            q_chunk(0)
            # fk/fv chunk pairs with i-chunk 0's attention chasing them.
            o_ps0 = [ps.tile([128, 512], F32, tag=f"o{cb}", bufs=1,
                             name=f"o{cb}_ps")
                     for cb in range(CB)]
            t1s0 = []
            for icn in range(IC):
                k_chunk(icn)
                p_t = b_s_exp(0, icn, o_ps0, t1s0)
                v_chunk(icn)
                b_pv(icn, o_ps0, p_t)
            b_tail(0, o_ps0, t1s0)

            q_chunk(1)
            for ic in range(1, IC):
                if ic + 1 < IC:
                    q_chunk(ic + 1)
                o_ps = [ps.tile([128, 512], F32, tag=f"o{cb}", bufs=1,
                                name=f"o{cb}_ps")
                        for cb in range(CB)]
                t1s = []
                for jg in range(JG):
                    p_t = b_s_exp(ic, jg, o_ps, t1s)
                    b_pv(jg, o_ps, p_t)
                b_tail(ic, o_ps, t1s)

    nc.compile()
    return nc


_NC_CACHE = []
LAST_RESULT = []  # last BassKernelResults, for perf inspection by test.py


def kernel(**inputs) -> np.ndarray:
    fq = np.ascontiguousarray(np.asarray(inputs["feature_q"], dtype=np.float32))
    fk = np.ascontiguousarray(np.asarray(inputs["feature_k"], dtype=np.float32))
    fv = np.ascontiguousarray(np.asarray(inputs["feature_v"], dtype=np.float32))
    wq = np.asarray(inputs["Wq"], dtype=np.float32)
    wk = np.asarray(inputs["Wk"], dtype=np.float32)
    wv = np.asarray(inputs["Wv"], dtype=np.float32)

    # weight layout prep (pure layout/scale folding, no heavy compute):
    # on-device pooling is a 2x2 *sum*; q,k each pick up 4x -> s is 16x,
    # folded into the on-device exp scale; v's 4x is folded into WvT here.
    wqt = np.ascontiguousarray(wq.T.astype(ml_dtypes.bfloat16))
    wkt = np.ascontiguousarray(wk.T.astype(ml_dtypes.bfloat16))
    wvt = np.ascontiguousarray(
        (wv.T * 0.25).astype(ml_dtypes.bfloat16))     # (C, C) [c_in, c_out]

    if not _NC_CACHE:
        _NC_CACHE.append(build_module())
    nc = _NC_CACHE[0]

    in_maps = [
        {
            "feature_q": fq[b],
            "feature_k": fk[b],
            "feature_v": fv[b],
            "WqT": wqt,
            "WkT": wkt,
            "WvT": wvt,
        }
        for b in range(B)
    ]
    res = run_bass_kernel_spmd(nc, in_maps, core_ids=list(range(B)))
    LAST_RESULT.clear()
    LAST_RESULT.append(res)
    out = np.stack([res.results[b]["out"] for b in range(B)], axis=0)
    return out.astype(np.float32)


if __name__ == "__main__":
    nc = build_module()
    print("module built + compiled OK")
